# revision 6
# baseline (speedup 1.0000x reference)
"""CaptioningRNN (LSTM + spatial attention + vocab loss) on 8 Trainium2 cores.

Strategy (v2):
 - Parallel matmul groups (CNN projection, x@Wx precompute, vocab logits +
   logsumexp) sharded 8 ways; the sequential LSTM sharded over the 4H gate
   dimension (core c owns a 128-wide slice of each gate, aligned with hidden
   slice c).  Each step ends with a fused AllGather carrying the core's
   transposed h-slice + its partial attention scores.
 - attn@Wattn eliminated: B2[(n,p),:] = A[n,:,p] @ Wattn is computed once from
   images via the host-precomputed WPA = W_proj @ Wattn (no A AllGather), and
   applied per step as a block-diagonal PE matmul whose weights are the
   softmax w, expanded to the (n,p) K-dim with one STT + one tiny matmul +
   one masked multiply (no serial DVE FMA chain).
 - All per-step activations are exp/tanh (sigmoid via 0.5*(1+tanh(x/2)) with
   a doubled-h representation folded into host weight scaling) so the ACT
   table is loaded once.
 - Vocab rows run at lag 2 so their matmuls fill the AllGather wait window.
 - Unpack of the gathered payload is 2 strided DMAs on the two HWDGE rings.
 - Loss: logsumexp partials per vocab shard, label logits via host-gathered
   W_vocab columns, one final AllGather, replicated tiny reduction.
   b_vocab[y] mask term added on host.
"""
import sys, os, time

sys.path.insert(0, "/opt/trn_rl_repo")

import numpy as np
import ml_dtypes

import concourse.bass as bass
import concourse.bacc as bacc
import concourse.mybir as mybir
import concourse.tile as tile
import concourse.bass_isa as bass_isa
from concourse.bass_utils import run_bass_kernel_spmd


BF16 = ml_dtypes.bfloat16
F32 = mybir.dt.float32
BF = mybir.dt.bfloat16
U8 = mybir.dt.uint8

NCORES = 8
N = 128          # batch
TCAP = 31
CIN = 1280
WD = 512         # embed dim
H = 1024         # hidden
V = 10000        # vocab
P = 16           # spatial positions
HS = H // NCORES      # 128 hidden slice
SC = 4 * HS           # 512 a-columns per core
VS = V // NCORES      # 1250 vocab slice
AluOp = mybir.AluOpType
Act = mybir.ActivationFunctionType

HT_BYTES = 128 * 128 * 2           # h^T slice, bf16
SC_BYTES = 2048 * 4                # partial scores, fp32
PAY = HT_BYTES + SC_BYTES          # per-step AG payload bytes


def build(T):
    nc = bacc.Bacc("TRN2", target_bir_lowering=False, debug=False,
                   num_devices=NCORES)
    NT = T * N

    def din(name, shape, dt):
        return nc.dram_tensor(name, shape, dt, kind="ExternalInput").ap()

    imgsT = din("imgsT", [CIN, N * P], BF)
    wprojS = din("wprojS", [CIN, HS], BF)
    bprojS = din("bprojS", [HS, 1], F32)
    xembT = din("xembT", [WD, NT], BF)
    WxS = din("WxS", [WD, SC], BF)
    bSb = din("bSb", [N, SC], F32)
    WhS = din("WhS", [H, SC], BF)          # pre-halved on host
    wpaS = din("wpaS", [CIN, SC], BF)      # W_proj @ Wattn slice
    bbrep = din("bbrep", [N, SC], F32)     # b_proj @ Wattn slice, replicated
    wvoc = din("wvoc", [H, VS], BF)        # pre-halved
    bvoc = din("bvoc", [1, VS], BF)
    wyT = din("wyT", [T, 128, H], BF)      # pre-halved
    mask_f = din("mask_f", [N, T], F32)
    ident = din("ident", [128, 128], BF)
    ones32 = din("ones32", [128, 32], BF)  # value 1/64 (scores scale, 2h rep)
    ones1 = din("ones1", [128, 1], BF)
    onesrow = din("onesrow", [1, 128], BF)
    maskt = din("maskt", [128, 128], BF)   # [n, r]: r//16 == n%8
    sel = din("sel", [128, 16], BF)        # [n, k]: n//8 == k
    mask32 = din("mask32", [128, 512], BF)  # [r, k*32+j]: j == 8*(k%4)+r//16

    loss_out = nc.dram_tensor("loss", [1, 1], F32, kind="ExternalOutput").ap()

    VCH = [(0, 512), (512, 512), (1024, VS - 1024)]  # vocab chunks
    ZB = N * T * 4
    rg = [list(range(NCORES))]

    with tile.TileContext(nc) as tc:
        with (
            tc.tile_pool(name="dram", bufs=1, space="DRAM") as dram,
            tc.tile_pool(name="persist", bufs=1) as pp,
            tc.tile_pool(name="work", bufs=3) as wp,
            tc.tile_pool(name="psB", bufs=1, space="PSUM") as psB,
        ):
            # ---------- persistent SBUF ----------
            atm = pp.tile([128, N, P], BF, name="atm")        # A^T my-slice (n,p)
            b2_sb = pp.tile([128, 16, SC], BF, name="b2_sb")  # B2 K-tiles
            whs_sb = pp.tile([128, 8, SC], BF, name="whs_sb")
            wvoc_sb = pp.tile([128, 8, VS], BF, name="wvoc_sb")
            bvoc_sb = pp.tile([1, VS], BF, name="bvoc_sb")
            xwx_sb = pp.tile([128, T, SC], BF, name="xwx_sb")
            zc_sb = pp.tile([128, T, 3], F32, name="zc_sb")

            c_sb = pp.tile([128, 128], F32, name="c_sb")
            ident_sb = pp.tile([128, 128], BF, name="ident_sb")
            ones32_sb = pp.tile([128, 32], BF, name="ones32_sb")
            ones1_sb = pp.tile([128, 1], BF, name="ones1_sb")
            onesrow_sb = pp.tile([1, 128], BF, name="onesrow_sb")
            maskt_sb = pp.tile([128, 128], BF, name="maskt_sb")
            sel_sb = pp.tile([128, 16], BF, name="sel_sb")
            mask32_sb = pp.tile([128, 16, 32], BF, name="mask32_sb")
            bproj_sb = pp.tile([HS, 1], F32, name="bproj_sb")
            mask_sb = pp.tile([N, T], F32, name="mask_sb")
            bbrep_sb = pp.tile([N, SC], F32, name="bbrep_sb")
            bsb_sb = pp.tile([128, SC], F32, name="bsb_sb")

            for dst, src in [(ident_sb, ident), (ones32_sb, ones32),
                             (ones1_sb, ones1), (onesrow_sb, onesrow),
                             (maskt_sb, maskt), (sel_sb, sel),
                             (bproj_sb, bprojS), (mask_sb, mask_f),
                             (bvoc_sb, bvoc), (bbrep_sb, bbrep),
                             (bsb_sb, bSb)]:
                nc.sync.dma_start(dst[:], src[:])
            nc.sync.dma_start(
                mask32_sb[:].rearrange("p a b -> p (a b)"), mask32[:])
            for kt in range(8):
                nc.sync.dma_start(whs_sb[:, kt, :], WhS[bass.ts(kt, 128), :])
                nc.sync.dma_start(wvoc_sb[:, kt, :], wvoc[bass.ts(kt, 128), :])

            # ---------- P1: projection  A^T[my hslice, (n,p)] ----------
            # two 1024-wide halves so the shared ps_big region is 2 banks
            wproj_sb = pp.tile([128, 10, 128], BF, name="wproj_sb")
            wpa_sb = pp.tile([128, 10, SC], BF, name="wpa_sb")
            for kt in range(10):
                nc.sync.dma_start(wproj_sb[:, kt, :], wprojS[bass.ts(kt, 128), :])
                nc.sync.dma_start(wpa_sb[:, kt, :], wpaS[bass.ts(kt, 128), :])
            imgs_sb = pp.tile([128, 10, N * P], BF, name="imgs_sb")
            for kt in range(10):
                nc.scalar.dma_start(imgs_sb[:, kt, :], imgsT[bass.ts(kt, 128), :])
            for hh in range(2):
                ps_a = psB.tile([128, 1024], F32, name=f"ps_p1_{hh}",
                                tag="ps_big", bufs=1)
                for kt in range(10):
                    for ch in range(2):
                        nc.tensor.matmul(ps_a[:, bass.ts(ch, 512)],
                                         wproj_sb[:, kt, :],
                                         imgs_sb[:, kt,
                                                 1024 * hh + 512 * ch:
                                                 1024 * hh + 512 * (ch + 1)],
                                         start=(kt == 0), stop=(kt == 9))
                nc.scalar.activation(
                    atm[:].rearrange("p n q -> p (n q)")[:, bass.ts(hh, 1024)],
                    ps_a[:], Act.Identity, bias=bproj_sb[:])
            # H0 = 2*h0 : sum over positions / 8
            h0t_f = pp.tile([128, 128], F32, name="h0t_f")
            nc.vector.tensor_reduce(h0t_f[:], atm[:], mybir.AxisListType.X,
                                    AluOp.add)
            nc.scalar.mul(h0t_f[:], h0t_f[:], 1.0 / 8.0)
            h0t_b = pp.tile([128, 128], BF, name="h0t_b")
            nc.scalar.copy(h0t_b[:], h0t_f[:])
            # C0 = H0 (doubled carry), batch-major my hidden slice
            ps_tr0 = psB.tile([128, 128], BF, name="ps_tr0", tag="ps_tr",
                              bufs=2)
            nc.tensor.transpose(ps_tr0[:], h0t_b[:], ident_sb[:])
            nc.scalar.copy(c_sb[:], ps_tr0[:])

            # scores partial + payload + AG
            def scores_and_ag(step, hT_bf):
                e_sb = wp.tile([128, N, P], BF, name=f"e_{step}", tag="e_sb",
                               bufs=2)
                nc.vector.tensor_mul(
                    e_sb[:, 0:64, :], atm[:, 0:64, :],
                    hT_bf[:, 0:64].unsqueeze(2).broadcast_to([128, 64, P]))
                nc.gpsimd.tensor_mul(
                    e_sb[:, 64:128, :], atm[:, 64:128, :],
                    hT_bf[:, 64:128].unsqueeze(2).broadcast_to([128, 64, P]))
                ps_sc = psB.tile([128, 512], F32, name=f"ps_sc{step}",
                                 tag="ps_sc32", bufs=1)
                ev = e_sb[:].rearrange("p n q -> p (n q)")
                for ch in range(4):
                    nc.tensor.matmul(ps_sc[32 * ch:32 * (ch + 1), :],
                                     ones32_sb[:],
                                     ev[:, bass.ts(ch, 512)],
                                     start=True, stop=True,
                                     tile_position=(0, 32 * ch))
                sc_out = wp.tile([128, 512], F32, name=f"sco{step}",
                                 tag="sc_out", bufs=2)
                nc.scalar.copy(sc_out[:], ps_sc[:])
                pay = dram.tile([PAY], U8, name=f"pay{step}")
                nc.sync.dma_start(
                    pay[0:HT_BYTES].rearrange("(p b) -> p b", p=128),
                    hT_bf[:].bitcast(mybir.dt.uint8))
                nc.scalar.dma_start(
                    pay[HT_BYTES:PAY].bitcast(F32)
                    .rearrange("(c f) -> c f", c=4),
                    sc_out[0:128:32, :])
                gat = dram.tile([NCORES, PAY], U8, name=f"gat{step}")
                nc.gpsimd.collective_compute(
                    "AllGather", AluOp.bypass, replica_groups=rg,
                    ins=[pay.opt()], outs=[gat.opt()])
                return gat

            gat = scores_and_ag(0, h0t_b)

            # ---------- B2 precompute: B2[(n,p), c] = A[n,:,p] @ Wattn ----
            # via WPA = W_proj @ Wattn (host): B2 = imgsT^T @ WPA + bb
            for mt in range(16):
                ps_b = psB.tile([128, SC], F32, name=f"ps_b{mt}", tag="ps_mm",
                                bufs=2)
                for kt in range(10):
                    nc.tensor.matmul(ps_b[:], imgs_sb[:, kt, bass.ts(mt, 128)],
                                     wpa_sb[:, kt, :],
                                     start=(kt == 0), stop=(kt == 9))
                nc.vector.tensor_add(b2_sb[:, mt, :], ps_b[:], bbrep_sb[:])

            # ---------- P2: xwx[t] = x_t @ WxS + b  (SBUF resident) --------
            wxs_sb = pp.tile([128, 4, SC], BF, name="wxs_sb")
            for kt in range(4):
                nc.sync.dma_start(wxs_sb[:, kt, :], WxS[bass.ts(kt, 128), :])
            for t in range(T):
                xt_sb = wp.tile([128, 4, 128], BF, name=f"xt{t}", tag="xt_sb",
                                bufs=2)
                for kt in range(4):
                    nc.scalar.dma_start(xt_sb[:, kt, :],
                                        xembT[bass.ts(kt, 128), bass.ts(t, 128)])
                ps_x = psB.tile([128, SC], F32, name=f"ps_x{t}", tag="ps_mm",
                                bufs=2)
                for kt in range(4):
                    nc.tensor.matmul(ps_x[:], xt_sb[:, kt, :], wxs_sb[:, kt, :],
                                     start=(kt == 0), stop=(kt == 3))
                nc.vector.tensor_add(xwx_sb[:, t, :], ps_x[:], bsb_sb[:])

            # ---------- recurrence ----------
            pay2 = dram.tile([N * T * 4 + T * 128 * 4], U8, name="pay2")

            def vocab_row(trow, hT_sb):
                # logsumexp partials + label logits for hs row `trow`
                for ci, (off, ln) in enumerate(VCH):
                    ps_v = psB.tile([128, 512], F32, name=f"ps_v{trow}_{ci}",
                                    tag="ps_mm", bufs=2)
                    for kt in range(8):
                        nc.tensor.matmul(ps_v[:, :ln], hT_sb[:, kt, :],
                                         wvoc_sb[:, kt, off:off + ln],
                                         start=(kt == 0), stop=False)
                    nc.tensor.matmul(ps_v[:, :ln], onesrow_sb[:],
                                     bvoc_sb[:, off:off + ln],
                                     start=False, stop=True)
                    ex_scr = wp.tile([128, 512], BF, name=f"ex{trow}_{ci}",
                                     tag="ex_scr", bufs=2)
                    nc.scalar.activation(ex_scr[:, :ln], ps_v[:, :ln], Act.Exp,
                                         accum_out=zc_sb[:, trow, ci:ci + 1])
                # label logit
                wyt_sb = wp.tile([128, H], BF, name=f"wyt{trow}", tag="wyt_sb",
                                 bufs=2)
                nc.gpsimd.dma_start(wyt_sb[:], wyT[trow])
                ey_sb = wp.tile([128, H], BF, name=f"ey{trow}", tag="ey_sb",
                                bufs=2)
                nc.vector.tensor_mul(
                    ey_sb[:].rearrange("p (n k) -> p n k", k=8),
                    hT_sb[:].transpose([0, 2, 1]),
                    wyt_sb[:].rearrange("p (n k) -> p n k", k=8))
                ps_ll = psB.tile([1, 1024], F32, name=f"ps_ll{trow}",
                                 tag="ps_big", bufs=1)
                for ch in range(2):
                    nc.tensor.matmul(ps_ll[:, bass.ts(ch, 512)], ones1_sb[:],
                                     ey_sb[:, bass.ts(ch, 512)],
                                     start=True, stop=True)
                ll_t = wp.tile([1, 128], F32, name=f"ll_t{trow}",
                               tag="ll_t", bufs=2)
                nc.vector.tensor_reduce(
                    ll_t[:],
                    ps_ll[:].rearrange("o (n k) -> o n k", k=8),
                    mybir.AxisListType.X, AluOp.add)
                nc.gpsimd.dma_start(
                    pay2[ZB:].bitcast(F32)
                    .rearrange("(n t) -> t n", n=128)[trow:trow + 1, :],
                    ll_t[:])

            hT_prev = None
            for t in range(T):
                # unpack AG(t): full h^T and score partials
                hT_sb = wp.tile([128, 8, 128], BF, name=f"hT{t}", tag="hT_sb",
                                bufs=3)
                nc.sync.dma_start(
                    hT_sb[:],
                    gat[:, 0:HT_BYTES].bitcast(BF)
                    .rearrange("c (p b) -> p c b", p=128))
                sc_sb = wp.tile([128, 8, P], F32, name=f"sc{t}", tag="sc_sb",
                                bufs=2)
                nc.scalar.dma_start(
                    sc_sb[:],
                    gat[:, HT_BYTES:PAY].bitcast(F32)
                    .rearrange("c (n q) -> n c q", n=128))

                # a-psum group: xwx (identity mm) + Wh + blockdiag attn
                ps_a = psB.tile([128, SC], F32, name=f"ps_a{t}", tag="ps_a",
                                bufs=1)
                nc.tensor.matmul(ps_a[:], ident_sb[:], xwx_sb[:, t, :],
                                 start=True, stop=False)

                # vocab row t-2 fills the AG wait window on PE
                if t >= 2:
                    vocab_row(t - 2, hT_prev)

                for kt in range(8):
                    nc.tensor.matmul(ps_a[:], hT_sb[:, kt, :], whs_sb[:, kt, :],
                                     start=False, stop=False)

                # softmax over positions (summed partials)
                ssum = wp.tile([128, P], F32, name=f"ssum{t}", tag="ssum")
                nc.vector.tensor_reduce(ssum[:],
                                        sc_sb[:].rearrange("n c q -> n q c"),
                                        mybir.AxisListType.X, AluOp.add)
                e_w = wp.tile([128, P], F32, name=f"ew{t}", tag="e_w")
                zs = wp.tile([128, 1], F32, name=f"zs{t}", tag="zs")
                nc.scalar.activation(e_w[:], ssum[:], Act.Exp, accum_out=zs[:])
                rz = wp.tile([128, 1], F32, name=f"rz{t}", tag="rz")
                nc.vector.reciprocal(rz[:], zs[:])
                # normalized w expanded to the (n,p) K-dim:
                # w_rep[n, r] = w[n, r%16] * maskt[n, r]
                w_rep = wp.tile([128, 128], BF, name=f"wrep{t}", tag="w_rep")
                nc.vector.scalar_tensor_tensor(
                    w_rep[:].rearrange("p (a q) -> p a q", q=P),
                    e_w[:].unsqueeze(1).broadcast_to([128, 8, P]),
                    rz[:], maskt_sb[:].rearrange("p (a q) -> p a q", q=P),
                    op0=AluOp.mult, op1=AluOp.mult)
                ps_w = psB.tile([128, 16], F32, name=f"ps_w{t}", tag="ps_tr",
                                bufs=2)
                nc.tensor.matmul(ps_w[:], w_rep[:], sel_sb[:],
                                 start=True, stop=True)
                w_blk = wp.tile([128, 16, 32], BF, name=f"wblk{t}", tag="w_blk")
                nc.vector.tensor_mul(
                    w_blk[:], mask32_sb[:],
                    ps_w[:].unsqueeze(2).broadcast_to([128, 16, 32]))
                for k in range(16):
                    nc.tensor.matmul(ps_a[32 * (k // 4):32 * (k // 4) + 32, :],
                                     w_blk[:, k, :], b2_sb[:, k, :],
                                     start=False, stop=(k == 15),
                                     tile_position=(0, 32 * (k // 4)))

                # gates (sigmoid via tanh(x/2); h kept doubled)
                t_i = wp.tile([128, 128], BF, name=f"ti{t}", tag="t_i")
                t_f = wp.tile([128, 128], BF, name=f"tf{t}", tag="t_f")
                t_o = wp.tile([128, 128], BF, name=f"to{t}", tag="t_o")
                t_g = wp.tile([128, 128], BF, name=f"tg{t}", tag="t_g")
                nc.scalar.activation(t_i[:], ps_a[:, 0:128], Act.Tanh, scale=0.5)
                nc.scalar.activation(t_f[:], ps_a[:, 128:256], Act.Tanh, scale=0.5)
                nc.scalar.activation(t_o[:], ps_a[:, 256:384], Act.Tanh, scale=0.5)
                nc.scalar.activation(t_g[:], ps_a[:, 384:512], Act.Tanh)
                # C' = ((1+t_f)C)/2 + (1+t_i)t_g   (C doubled)
                u_t = wp.tile([128, 128], F32, name=f"u{t}", tag="u_t")
                v_t = wp.tile([128, 128], F32, name=f"v{t}", tag="v_t")
                nc.vector.scalar_tensor_tensor(u_t[:], t_f[:], 1.0, c_sb[:],
                                               op0=AluOp.add, op1=AluOp.mult)
                nc.vector.scalar_tensor_tensor(v_t[:], t_i[:], 1.0, t_g[:],
                                               op0=AluOp.add, op1=AluOp.mult)
                nc.vector.scalar_tensor_tensor(c_sb[:], u_t[:], 0.5, v_t[:],
                                               op0=AluOp.mult, op1=AluOp.add)
                tc_t = wp.tile([128, 128], BF, name=f"tc{t}", tag="tc_t")
                nc.scalar.activation(tc_t[:], c_sb[:], Act.Tanh, scale=0.5)
                h_sl = wp.tile([128, 128], BF, name=f"hsl{t}", tag="h_sl")
                nc.vector.scalar_tensor_tensor(h_sl[:], t_o[:], 1.0, tc_t[:],
                                               op0=AluOp.add, op1=AluOp.mult)
                # transpose h slice
                ps_tr = psB.tile([128, 128], BF, name=f"ps_tr{t}",
                                 tag="ps_tr", bufs=2)
                nc.tensor.transpose(ps_tr[:], h_sl[:], ident_sb[:])
                hT_c = wp.tile([128, 128], BF, name=f"hTc{t}", tag="hT_c")
                nc.scalar.copy(hT_c[:], ps_tr[:])
                # scores partial for h_{t+1} + AllGather
                gat = scores_and_ag(t + 1, hT_c)
                hT_prev = hT_sb

            # tail vocab rows
            vocab_row(T - 2, hT_prev)
            hT_last = wp.tile([128, 8, 128], BF, name="hT_last", tag="hT_sb")
            nc.sync.dma_start(
                hT_last[:],
                gat[:, 0:HT_BYTES].bitcast(BF)
                .rearrange("c (p b) -> p c b", p=128))
            vocab_row(T - 1, hT_last)

            # ---------- final loss ----------
            zfin = wp.tile([128, T], F32, name="zfin")
            nc.vector.tensor_reduce(zfin[:], zc_sb[:], mybir.AxisListType.X,
                                    AluOp.add)
            nc.sync.dma_start(
                pay2[0:ZB].bitcast(F32).rearrange("(p b) -> p b", p=128),
                zfin[:])
            gat2 = dram.tile([NCORES, N * T * 4 + T * 128 * 4], U8, name="gat2")
            nc.gpsimd.collective_compute(
                "AllGather", AluOp.bypass, replica_groups=rg,
                ins=[pay2.opt()], outs=[gat2.opt()])
            zg = wp.tile([128, T, 8], F32, name="zg")
            lg = wp.tile([128, T, 8], F32, name="lg")
            for c2 in range(8):
                nc.sync.dma_start(
                    zg[:, :, c2],
                    gat2[c2, 0:ZB].bitcast(F32).rearrange("(n q) -> n q", n=128))
                nc.scalar.dma_start(
                    lg[:, :, c2],
                    gat2[c2, ZB:].bitcast(F32)
                    .rearrange("(n q) -> n q", n=128))
            zred = wp.tile([128, T], F32, name="zred")
            llred = wp.tile([128, T], F32, name="llred")
            nc.vector.tensor_reduce(zred[:], zg[:], mybir.AxisListType.X,
                                    AluOp.add)
            nc.vector.tensor_reduce(llred[:], lg[:], mybir.AxisListType.X,
                                    AluOp.add)
            lse = wp.tile([128, T], F32, name="lse")
            nc.scalar.activation(lse[:], zred[:], Act.Ln)
            diff = wp.tile([128, T], F32, name="diff")
            nc.vector.tensor_sub(diff[:], lse[:], llred[:])
            nc.vector.tensor_mul(diff[:], diff[:], mask_sb[:])
            per_n = wp.tile([128, 1], F32, name="per_n")
            nc.vector.tensor_reduce(per_n[:], diff[:], mybir.AxisListType.X,
                                    AluOp.add)
            pn_red = wp.tile([128, 1], F32, name="pn_red")
            nc.gpsimd.partition_all_reduce(pn_red[:], per_n[:], 128,
                                           bass_isa.ReduceOp.add)
            loss_sb = wp.tile([1, 1], F32, name="loss_sb")
            nc.scalar.mul(loss_sb[:], pn_red[0:1, :], 1.0 / N)
            nc.sync.dma_start(loss_out[:], loss_sb[:])

    nc.compile()
    return nc


def host_prep(inputs, T):
    """Build the 8 per-core input maps (all numpy)."""
    g = {k: np.asarray(v) for k, v in inputs.items()}
    images, captions = g["images"], g["captions"]
    W_embed, W_proj, b_proj = g["W_embed"], g["W_proj"], g["b_proj"]
    Wx, Wh, Wattn, b = g["Wx"], g["Wh"], g["Wattn"], g["b"]
    W_vocab, b_vocab = g["W_vocab"], g["b_vocab"]

    cap = np.asarray(captions)
    cap_in = cap[:, :T]
    cap_out = cap[:, 1:T + 1]
    x_emb = W_embed[cap_in]                      # [N, T, WD]
    xembT = np.ascontiguousarray(
        x_emb.transpose(2, 1, 0).reshape(WD, T * N)).astype(BF16)
    imgsT = np.ascontiguousarray(
        images.reshape(N, CIN, P).transpose(1, 0, 2).reshape(CIN, N * P)
    ).astype(BF16)
    mask = (cap_out != 0).astype(np.float32)     # [N, T]
    ident = np.eye(128, dtype=BF16)
    ones32 = np.full((128, 32), 1.0 / 64.0, dtype=BF16)
    ones1 = np.ones((128, 1), dtype=BF16)
    onesrow = np.ones((1, 128), dtype=BF16)

    # attn projection fused with W_proj (B2 = imgs^T @ WPA + bb)
    WPA = (W_proj.astype(np.float32) @ Wattn.astype(np.float32))  # [CIN, 4H]
    bb = (b_proj.astype(np.float32) @ Wattn.astype(np.float32))   # [4H]

    # block-diag build constants
    rr = np.arange(128)
    nn_i = np.arange(128)
    maskt = (rr[None, :] // 16 == nn_i[:, None] % 8).astype(BF16)  # [n, r]
    kk = np.arange(16)
    sel = (nn_i[:, None] // 8 == kk[None, :]).astype(BF16)         # [n, k]
    jj = np.arange(32)
    mask32 = (jj[None, None, :] == 8 * (kk[None, :, None] % 4)
              + rr[:, None, None] // 16).astype(BF16)              # [r, k, j]
    mask32 = np.ascontiguousarray(mask32.reshape(128, 512))

    # label weight vectors, [H, N, T] -> per t: [hl, (n, kt)] (pre-halved)
    wy = W_vocab[:, cap_out] * 0.5               # [H, N, T]
    wy_t = wy.reshape(8, 128, N, T).transpose(3, 1, 2, 0)  # [T, hl, n, kt]

    in_maps = []
    for c in range(NCORES):
        hsl = slice(128 * c, 128 * (c + 1))
        idx = np.concatenate([g4 * H + 128 * c + np.arange(128)
                              for g4 in range(4)])
        vsl = slice(VS * c, VS * (c + 1))
        wyc = wy_t.copy()
        nm = np.zeros(N, dtype=wy_t.dtype)
        nm[16 * c:16 * (c + 1)] = 1
        wyc *= nm[None, None, :, None]
        in_maps.append({
            "imgsT": imgsT,
            "wprojS": np.ascontiguousarray(W_proj[:, hsl]).astype(BF16),
            "bprojS": np.ascontiguousarray(b_proj[hsl, None]).astype(np.float32),
            "xembT": xembT,
            "WxS": np.ascontiguousarray(Wx[:, idx]).astype(BF16),
            "bSb": np.ascontiguousarray(
                np.broadcast_to(b[idx], (N, SC))).astype(np.float32),
            "WhS": np.ascontiguousarray(Wh[:, idx] * 0.5).astype(BF16),
            "wpaS": np.ascontiguousarray(WPA[:, idx]).astype(BF16),
            "bbrep": np.ascontiguousarray(
                np.broadcast_to(bb[idx], (N, SC))).astype(np.float32),
            "wvoc": np.ascontiguousarray(W_vocab[:, vsl] * 0.5).astype(BF16),
            "bvoc": np.ascontiguousarray(b_vocab[None, vsl]).astype(BF16),
            "wyT": np.ascontiguousarray(
                wyc.reshape(T, 128, H)).astype(BF16),
            "mask_f": mask,
            "ident": ident,
            "ones32": ones32,
            "ones1": ones1,
            "onesrow": onesrow,
            "maskt": maskt,
            "sel": sel,
            "mask32": mask32,
        })
    host_by = float(np.sum(mask.astype(np.float64) *
                           np.asarray(b_vocab, np.float64)[cap_out]) / N)
    return in_maps, host_by


_CACHE = {}


def _get_built(T):
    if T not in _CACHE:
        _CACHE[T] = build(T)
    return _CACHE[T]


def run(inputs, T=30):
    nc = _get_built(T)
    in_maps, host_by = host_prep(inputs, T)
    res = run_bass_kernel_spmd(nc, in_maps, core_ids=list(range(NCORES)))
    dev_loss = float(res.results[0]["loss"][0, 0])
    return np.float32(dev_loss - host_by)


def kernel(**inputs) -> np.ndarray:
    return run(inputs, T=30)


# revision 27
# speedup vs baseline: 80.2042x; 80.2042x over previous
"""CaptioningRNN (LSTM + spatial attention + vocab loss) on 8 Trainium2 cores.

Strategy (v2):
 - Parallel matmul groups (CNN projection, x@Wx precompute, vocab logits +
   logsumexp) sharded 8 ways; the sequential LSTM sharded over the 4H gate
   dimension (core c owns a 128-wide slice of each gate, aligned with hidden
   slice c).  Each step ends with a fused AllGather carrying the core's
   transposed h-slice + its partial attention scores.
 - attn@Wattn eliminated: B2[(n,p),:] = A[n,:,p] @ Wattn is computed once from
   images via the host-precomputed WPA = W_proj @ Wattn (no A AllGather), and
   applied per step as a block-diagonal PE matmul whose weights are the
   softmax w, expanded to the (n,p) K-dim with one STT + one tiny matmul +
   one masked multiply (no serial DVE FMA chain).
 - All per-step activations are exp/tanh (sigmoid via 0.5*(1+tanh(x/2)) with
   a doubled-h representation folded into host weight scaling) so the ACT
   table is loaded once.
 - Vocab rows run at lag 2 so their matmuls fill the AllGather wait window.
 - Unpack of the gathered payload is 2 strided DMAs on the two HWDGE rings.
 - Loss: logsumexp partials per vocab shard, label logits via host-gathered
   W_vocab columns, one final AllGather, replicated tiny reduction.
   b_vocab[y] mask term added on host.
"""
import sys, os, time

sys.path.insert(0, "/opt/trn_rl_repo")

import numpy as np
import ml_dtypes

import concourse.bass as bass
import concourse.bacc as bacc
import concourse.mybir as mybir
import concourse.tile as tile
import concourse.bass_isa as bass_isa
from concourse.bass_utils import run_bass_kernel_spmd


BF16 = ml_dtypes.bfloat16
F32 = mybir.dt.float32
BF = mybir.dt.bfloat16
U8 = mybir.dt.uint8

NCORES = 8
N = 128          # batch
TCAP = 31
CIN = 1280
WD = 512         # embed dim
H = 1024         # hidden
V = 10000        # vocab
P = 16           # spatial positions
HS = H // NCORES      # 128 hidden slice
SC = 4 * HS           # 512 a-columns per core
VS = V // NCORES      # 1250 vocab slice
AluOp = mybir.AluOpType
Act = mybir.ActivationFunctionType

HT_BYTES = 128 * 128 * 2           # h^T slice, bf16
SC_BYTES = 128 * 16 * 2            # partial scores, bf16
PAY = HT_BYTES + SC_BYTES          # per-step AG payload bytes


def build(T):
    nc = bacc.Bacc("TRN2", target_bir_lowering=False, debug=False,
                   num_devices=NCORES)
    NT = T * N

    def din(name, shape, dt):
        return nc.dram_tensor(name, shape, dt, kind="ExternalInput").ap()

    imgsT = din("imgsT", [CIN, N * P], BF)
    wprojS = din("wprojS", [CIN, HS], BF)
    bprojR = din("bprojR", [N, HS], F32)   # b_proj slice, row-replicated
    xembT = din("xembT", [WD, NT], BF)
    WxS = din("WxS", [WD, SC], BF)
    bSb = din("bSb", [N, SC], F32)
    WhS = din("WhS", [H, SC], BF)          # pre-halved on host
    wpaS = din("wpaS", [CIN, SC], BF)      # W_proj @ Wattn slice
    bbrep = din("bbrep", [N, SC], F32)     # b_proj @ Wattn slice, replicated
    wvoc = din("wvoc", [H, VS], BF)        # pre-halved
    bvoc = din("bvoc", [1, VS], BF)
    wyT = din("wyT", [T, 128, H], BF)      # pre-halved
    mask_f = din("mask_f", [N, T], F32)
    ident = din("ident", [128, 128], BF)
    ones1 = din("ones1", [128, 1], BF)
    onesrow = din("onesrow", [1, 128], BF)
    maskt = din("maskt", [128, 128], BF)   # [n, r]: r//16 == n%8
    sel = din("sel", [128, 16], BF)        # [n, k]: n//8 == k
    mask32 = din("mask32", [128, 512], BF)  # [r, k*32+j]: j == 8*(k%4)+r//16

    loss_out = nc.dram_tensor("loss", [1, 1], F32, kind="ExternalOutput").ap()

    VCH = [(0, 512), (512, 512), (1024, VS - 1024)]  # vocab chunks
    ZB = N * T * 4
    rg = [list(range(NCORES))]

    with tile.TileContext(nc) as tc:
        with (
            tc.tile_pool(name="dram", bufs=1, space="DRAM") as dram,
            tc.tile_pool(name="persist", bufs=1) as pp,
            tc.tile_pool(name="work", bufs=3) as wp,
            tc.tile_pool(name="psB", bufs=1, space="PSUM") as psB,
        ):
            # ---------- persistent SBUF ----------
            abm = pp.tile([128, P, HS], BF, name="abm")       # A batch-major slice
            b2_sb = pp.tile([128, 16, SC], BF, name="b2_sb")  # B2 K-tiles
            whs_sb = pp.tile([128, 8, SC], BF, name="whs_sb")
            wvoc_sb = pp.tile([128, 8, VS], BF, name="wvoc_sb")
            bvoc_sb = pp.tile([1, VS], BF, name="bvoc_sb")
            xwx_sb = pp.tile([128, T, SC], BF, name="xwx_sb")
            zc_sb = pp.tile([128, T, 3], F32, name="zc_sb")

            c_sb = pp.tile([128, 128], F32, name="c_sb")
            ident_sb = pp.tile([128, 128], BF, name="ident_sb")
            ones1_sb = pp.tile([128, 1], BF, name="ones1_sb")
            onesrow_sb = pp.tile([1, 128], BF, name="onesrow_sb")
            maskt_sb = pp.tile([128, 128], BF, name="maskt_sb")
            sel_sb = pp.tile([128, 16], BF, name="sel_sb")
            mask32_sb = pp.tile([128, 16, 32], BF, name="mask32_sb")
            bprojr_sb = pp.tile([N, HS], F32, name="bprojr_sb")
            mask_sb = pp.tile([N, T], F32, name="mask_sb")
            bbrep_sb = pp.tile([N, SC], F32, name="bbrep_sb")
            bsb_sb = pp.tile([128, SC], F32, name="bsb_sb")

            for dst, src in [(ident_sb, ident),
                             (ones1_sb, ones1), (onesrow_sb, onesrow),
                             (maskt_sb, maskt), (sel_sb, sel),
                             (bprojr_sb, bprojR), (mask_sb, mask_f),
                             (bvoc_sb, bvoc), (bbrep_sb, bbrep),
                             (bsb_sb, bSb)]:
                nc.sync.dma_start(dst[:], src[:])
            nc.sync.dma_start(
                mask32_sb[:].rearrange("p a b -> p (a b)"), mask32[:])
            for kt in range(8):
                nc.sync.dma_start(whs_sb[:, kt, :], WhS[bass.ts(kt, 128), :])
                nc.sync.dma_start(wvoc_sb[:, kt, :], wvoc[bass.ts(kt, 128), :])

            # ---------- P1: projection, batch-major  A[n, p, my hslice] ----
            wproj_sb = pp.tile([128, 10, 128], BF, name="wproj_sb")
            wpa_sb = pp.tile([128, 10, SC], BF, name="wpa_sb")
            for kt in range(10):
                nc.sync.dma_start(wproj_sb[:, kt, :], wprojS[bass.ts(kt, 128), :])
                nc.sync.dma_start(wpa_sb[:, kt, :], wpaS[bass.ts(kt, 128), :])
            imgs_sb = pp.tile([128, 10, N, P], BF, name="imgs_sb")
            for kt in range(10):
                nc.scalar.dma_start(
                    imgs_sb[:, kt, :, :].rearrange("p n q -> p (n q)"),
                    imgsT[bass.ts(kt, 128), :])
            for p in range(P):
                ps_p = psB.tile([128, 128], F32, name=f"ps_p1_{p}",
                                tag="ps_mm", bufs=2)
                for kt in range(10):
                    nc.tensor.matmul(ps_p[:], imgs_sb[:, kt, :, p],
                                     wproj_sb[:, kt, :],
                                     start=(kt == 0), stop=(kt == 9))
                nc.vector.tensor_add(abm[:, p, :], ps_p[:], bprojr_sb[:])
            # H0 = 2*h0 : sum over positions / 8  (batch-major)
            h0_f = pp.tile([128, 128], F32, name="h0_f")
            nc.vector.tensor_reduce(h0_f[:],
                                    abm[:].rearrange("n q h -> n h q"),
                                    mybir.AxisListType.X, AluOp.add)
            h0_b = pp.tile([128, 128], BF, name="h0_b")
            nc.scalar.mul(h0_b[:], h0_f[:], 1.0 / 8.0)
            nc.scalar.mul(c_sb[:], h0_f[:], 1.0 / 8.0)  # C0 = H0

            # transpose h-slice, payload (h^T + batch-major score partials), AG
            def scores_and_ag(step, h_bf):
                ps_tr = psB.tile([128, 128], BF, name=f"ps_tr{step}",
                                 tag="ps_tr", bufs=2)
                nc.tensor.transpose(ps_tr[:], h_bf[:], ident_sb[:])
                hT_c = wp.tile([128, 128], BF, name=f"hTc{step}", tag="hT_c",
                               bufs=2)
                nc.scalar.copy(hT_c[:], ps_tr[:])
                pay = dram.tile([PAY], U8, name=f"pay{step}")
                nc.sync.dma_start(
                    pay[0:HT_BYTES].rearrange("(p b) -> p b", p=128),
                    hT_c[:].bitcast(mybir.dt.uint8))
                # partial scores (unscaled): sc[n, p] = sum_h A[n,p,h]*H[n,h]
                e_bm = wp.tile([128, P, HS], BF, name=f"e{step}", tag="e_bm",
                               bufs=2)
                sc_p = wp.tile([128, P], BF, name=f"scp{step}", tag="sc_p",
                               bufs=2)
                nc.vector.tensor_mul(
                    e_bm[:], abm[:],
                    h_bf[:].unsqueeze(1).broadcast_to([128, P, HS]))
                with nc.allow_low_precision(
                        reason="bf16 score partials; fp32 internal accum"):
                    nc.vector.tensor_reduce(sc_p[:], e_bm[:],
                                            mybir.AxisListType.X, AluOp.add)
                nc.scalar.dma_start(
                    pay[HT_BYTES:PAY].rearrange("(p b) -> p b", p=128),
                    sc_p[:].bitcast(mybir.dt.uint8))
                gat = dram.tile([NCORES, PAY], U8, name=f"gat{step}")
                nc.gpsimd.collective_compute(
                    "AllGather", AluOp.bypass, replica_groups=rg,
                    ins=[pay.opt()], outs=[gat.opt()])
                return gat

            gat = scores_and_ag(0, h0_b)

            # ---------- B2 precompute: B2[(n,p), c] = A[n,:,p] @ Wattn ----
            # via WPA = W_proj @ Wattn (host): B2 = imgsT^T @ WPA + bb
            imgs_flat = imgs_sb[:].rearrange("p k n q -> p k (n q)")
            for mt in range(16):
                ps_b = psB.tile([128, SC], F32, name=f"ps_b{mt}", tag="ps_mm",
                                bufs=2)
                for kt in range(10):
                    nc.tensor.matmul(ps_b[:], imgs_flat[:, kt, bass.ts(mt, 128)],
                                     wpa_sb[:, kt, :],
                                     start=(kt == 0), stop=(kt == 9))
                nc.vector.tensor_add(b2_sb[:, mt, :], ps_b[:], bbrep_sb[:])

            # ---------- P2: xwx[t] = x_t @ WxS + b  (SBUF resident) --------
            wxs_sb = pp.tile([128, 4, SC], BF, name="wxs_sb")
            for kt in range(4):
                nc.sync.dma_start(wxs_sb[:, kt, :], WxS[bass.ts(kt, 128), :])
            for t in range(T):
                xt_sb = wp.tile([128, 4, 128], BF, name=f"xt{t}", tag="xt_sb",
                                bufs=2)
                for kt in range(4):
                    nc.scalar.dma_start(xt_sb[:, kt, :],
                                        xembT[bass.ts(kt, 128), bass.ts(t, 128)])
                ps_x = psB.tile([128, SC], F32, name=f"ps_x{t}", tag="ps_mm",
                                bufs=2)
                for kt in range(4):
                    nc.tensor.matmul(ps_x[:], xt_sb[:, kt, :], wxs_sb[:, kt, :],
                                     start=(kt == 0), stop=(kt == 3))
                nc.vector.tensor_add(xwx_sb[:, t, :], ps_x[:], bsb_sb[:])

            # ---------- recurrence ----------
            pay2 = dram.tile([N * T * 4 + T * 128 * 4], U8, name="pay2")

            def vocab_row(trow, hT_sb):
                # logsumexp partials + label logits for hs row `trow`
                for ci, (off, ln) in enumerate(VCH):
                    ps_v = psB.tile([128, 512], F32, name=f"ps_v{trow}_{ci}",
                                    tag="ps_mm", bufs=2)
                    for kt in range(8):
                        nc.tensor.matmul(ps_v[:, :ln], hT_sb[:, kt, :],
                                         wvoc_sb[:, kt, off:off + ln],
                                         start=(kt == 0), stop=False)
                    nc.tensor.matmul(ps_v[:, :ln], onesrow_sb[:],
                                     bvoc_sb[:, off:off + ln],
                                     start=False, stop=True)
                    ex_scr = wp.tile([128, 512], BF, name=f"ex{trow}_{ci}",
                                     tag="ex_scr", bufs=2)
                    nc.scalar.activation(ex_scr[:, :ln], ps_v[:, :ln], Act.Exp,
                                         accum_out=zc_sb[:, trow, ci:ci + 1])
                # label logit
                wyt_sb = wp.tile([128, H], BF, name=f"wyt{trow}", tag="wyt_sb",
                                 bufs=2)
                nc.gpsimd.dma_start(wyt_sb[:], wyT[trow])
                ey_sb = wp.tile([128, H], BF, name=f"ey{trow}", tag="ey_sb",
                                bufs=2)
                nc.gpsimd.tensor_mul(
                    ey_sb[:].rearrange("p (n k) -> p n k", k=8),
                    hT_sb[:].transpose([0, 2, 1]),
                    wyt_sb[:].rearrange("p (n k) -> p n k", k=8))
                ps_ll = psB.tile([1, 1024], F32, name=f"ps_ll{trow}",
                                 tag="ps_big", bufs=1)
                for ch in range(2):
                    nc.tensor.matmul(ps_ll[:, bass.ts(ch, 512)], ones1_sb[:],
                                     ey_sb[:, bass.ts(ch, 512)],
                                     start=True, stop=True)
                ll_t = wp.tile([1, 128], F32, name=f"ll_t{trow}",
                               tag="ll_t", bufs=2)
                nc.vector.tensor_reduce(
                    ll_t[:],
                    ps_ll[:].rearrange("o (n k) -> o n k", k=8),
                    mybir.AxisListType.X, AluOp.add)
                nc.gpsimd.dma_start(
                    pay2[ZB:].bitcast(F32)
                    .rearrange("(n t) -> t n", n=128)[trow:trow + 1, :],
                    ll_t[:])

            hT_prev = None
            for t in range(T):
                # unpack AG(t): score partials (scalar ring) + full h^T (sync)
                hT_sb = wp.tile([128, 8, 128], BF, name=f"hT{t}", tag="hT_sb",
                                bufs=3)
                sc_sb = wp.tile([128, 8, P], BF, name=f"sc{t}", tag="sc_sb",
                                bufs=2)
                nc.scalar.dma_start(
                    sc_sb[:],
                    gat[:, HT_BYTES:PAY].bitcast(BF)
                    .rearrange("c (n q) -> n c q", n=128))
                nc.sync.dma_start(
                    hT_sb[:],
                    gat[:, 0:HT_BYTES].bitcast(BF)
                    .rearrange("c (p b) -> p c b", p=128))

                # a-psum group: xwx (identity mm) + Wh + blockdiag attn
                ps_a = psB.tile([128, SC], F32, name=f"ps_a{t}", tag="ps_a",
                                bufs=2)
                nc.tensor.matmul(ps_a[:], ident_sb[:], xwx_sb[:, t, :],
                                 start=True, stop=False)

                for kt in range(8):
                    nc.tensor.matmul(ps_a[:], hT_sb[:, kt, :], whs_sb[:, kt, :],
                                     start=False, stop=False)

                # softmax over positions (summed partials)
                ssum = wp.tile([128, P], F32, name=f"ssum{t}", tag="ssum")
                nc.vector.tensor_reduce(ssum[:],
                                        sc_sb[:].rearrange("n c q -> n q c"),
                                        mybir.AxisListType.X, AluOp.add)
                e_w = wp.tile([128, P], F32, name=f"ew{t}", tag="e_w")
                zs = wp.tile([128, 1], F32, name=f"zs{t}", tag="zs")
                nc.scalar.activation(e_w[:], ssum[:], Act.Exp, scale=1.0 / 64.0,
                                     accum_out=zs[:])
                rz = wp.tile([128, 1], F32, name=f"rz{t}", tag="rz")
                nc.vector.reciprocal(rz[:], zs[:])
                # normalized w expanded to the (n,p) K-dim:
                # w_rep[n, r] = w[n, r%16] * maskt[n, r]
                w_rep = wp.tile([128, 128], BF, name=f"wrep{t}", tag="w_rep")
                nc.vector.scalar_tensor_tensor(
                    w_rep[:].rearrange("p (a q) -> p a q", q=P),
                    e_w[:].unsqueeze(1).broadcast_to([128, 8, P]),
                    rz[:], maskt_sb[:].rearrange("p (a q) -> p a q", q=P),
                    op0=AluOp.mult, op1=AluOp.mult)
                ps_w = psB.tile([128, 16], F32, name=f"ps_w{t}", tag="ps_tr",
                                bufs=2)
                nc.tensor.matmul(ps_w[:], w_rep[:], sel_sb[:],
                                 start=True, stop=True)
                wexp = wp.tile([128, 16], BF, name=f"wexp{t}", tag="wexp")
                nc.scalar.copy(wexp[:], ps_w[:])
                w_blk = wp.tile([128, 16, 32], BF, name=f"wblk{t}", tag="w_blk")
                nc.vector.tensor_mul(
                    w_blk[:], mask32_sb[:],
                    wexp[:].unsqueeze(2).broadcast_to([128, 16, 32]))
                for k in range(16):
                    nc.tensor.matmul(ps_a[32 * (k // 4):32 * (k // 4) + 32, :],
                                     w_blk[:, k, :], b2_sb[:, k, :],
                                     start=False, stop=(k == 15),
                                     tile_position=(0, 32 * (k // 4)))

                # gates (sigmoid via tanh(x/2); h kept doubled)
                t_i = wp.tile([128, 128], BF, name=f"ti{t}", tag="t_i")
                t_f = wp.tile([128, 128], BF, name=f"tf{t}", tag="t_f")
                t_o = wp.tile([128, 128], BF, name=f"to{t}", tag="t_o")
                t_g = wp.tile([128, 128], BF, name=f"tg{t}", tag="t_g")
                nc.scalar.activation(t_i[:], ps_a[:, 0:128], Act.Tanh, scale=0.5)
                nc.scalar.activation(t_f[:], ps_a[:, 128:256], Act.Tanh, scale=0.5)
                nc.scalar.activation(t_o[:], ps_a[:, 256:384], Act.Tanh, scale=0.5)
                nc.scalar.activation(t_g[:], ps_a[:, 384:512], Act.Tanh)
                # C' = ((1+t_f)C)/2 + (1+t_i)t_g   (C doubled)
                u_t = wp.tile([128, 128], F32, name=f"u{t}", tag="u_t")
                v_t = wp.tile([128, 128], F32, name=f"v{t}", tag="v_t")
                nc.vector.scalar_tensor_tensor(u_t[:], t_f[:], 1.0, c_sb[:],
                                               op0=AluOp.add, op1=AluOp.mult)
                nc.vector.scalar_tensor_tensor(v_t[:], t_i[:], 1.0, t_g[:],
                                               op0=AluOp.add, op1=AluOp.mult)
                nc.vector.scalar_tensor_tensor(c_sb[:], u_t[:], 0.5, v_t[:],
                                               op0=AluOp.mult, op1=AluOp.add)
                tc_t = wp.tile([128, 128], BF, name=f"tc{t}", tag="tc_t")
                nc.scalar.activation(tc_t[:], c_sb[:], Act.Tanh, scale=0.5)
                h_sl = wp.tile([128, 128], BF, name=f"hsl{t}", tag="h_sl")
                nc.vector.scalar_tensor_tensor(h_sl[:], t_o[:], 1.0, tc_t[:],
                                               op0=AluOp.add, op1=AluOp.mult)
                # transpose + payload + scores partial for h_{t+1} + AllGather
                gat = scores_and_ag(t + 1, h_sl)
                # vocab row t-2 issued last: every engine reaches it right
                # before the AG wait, so its work fills the AG window
                if t >= 2:
                    vocab_row(t - 2, hT_prev)
                hT_prev = hT_sb

            # tail vocab rows
            vocab_row(T - 2, hT_prev)
            hT_last = wp.tile([128, 8, 128], BF, name="hT_last", tag="hT_sb")
            nc.sync.dma_start(
                hT_last[:],
                gat[:, 0:HT_BYTES].bitcast(BF)
                .rearrange("c (p b) -> p c b", p=128))
            vocab_row(T - 1, hT_last)

            # ---------- final loss ----------
            zfin = wp.tile([128, T], F32, name="zfin")
            nc.vector.tensor_reduce(zfin[:], zc_sb[:], mybir.AxisListType.X,
                                    AluOp.add)
            nc.sync.dma_start(
                pay2[0:ZB].bitcast(F32).rearrange("(p b) -> p b", p=128),
                zfin[:])
            gat2 = dram.tile([NCORES, N * T * 4 + T * 128 * 4], U8, name="gat2")
            nc.gpsimd.collective_compute(
                "AllGather", AluOp.bypass, replica_groups=rg,
                ins=[pay2.opt()], outs=[gat2.opt()])
            zg = wp.tile([128, T, 8], F32, name="zg")
            lg = wp.tile([128, T, 8], F32, name="lg")
            for c2 in range(8):
                nc.sync.dma_start(
                    zg[:, :, c2],
                    gat2[c2, 0:ZB].bitcast(F32).rearrange("(n q) -> n q", n=128))
                nc.scalar.dma_start(
                    lg[:, :, c2],
                    gat2[c2, ZB:].bitcast(F32)
                    .rearrange("(n q) -> n q", n=128))
            zred = wp.tile([128, T], F32, name="zred")
            llred = wp.tile([128, T], F32, name="llred")
            nc.vector.tensor_reduce(zred[:], zg[:], mybir.AxisListType.X,
                                    AluOp.add)
            nc.vector.tensor_reduce(llred[:], lg[:], mybir.AxisListType.X,
                                    AluOp.add)
            lse = wp.tile([128, T], F32, name="lse")
            nc.scalar.activation(lse[:], zred[:], Act.Ln)
            diff = wp.tile([128, T], F32, name="diff")
            nc.vector.tensor_sub(diff[:], lse[:], llred[:])
            nc.vector.tensor_mul(diff[:], diff[:], mask_sb[:])
            per_n = wp.tile([128, 1], F32, name="per_n")
            nc.vector.tensor_reduce(per_n[:], diff[:], mybir.AxisListType.X,
                                    AluOp.add)
            pn_red = wp.tile([128, 1], F32, name="pn_red")
            nc.gpsimd.partition_all_reduce(pn_red[:], per_n[:], 128,
                                           bass_isa.ReduceOp.add)
            loss_sb = wp.tile([1, 1], F32, name="loss_sb")
            nc.scalar.mul(loss_sb[:], pn_red[0:1, :], 1.0 / N)
            nc.sync.dma_start(loss_out[:], loss_sb[:])

    nc.compile()
    return nc


def host_prep(inputs, T):
    """Build the 8 per-core input maps (all numpy)."""
    g = {k: np.asarray(v) for k, v in inputs.items()}
    images, captions = g["images"], g["captions"]
    W_embed, W_proj, b_proj = g["W_embed"], g["W_proj"], g["b_proj"]
    Wx, Wh, Wattn, b = g["Wx"], g["Wh"], g["Wattn"], g["b"]
    W_vocab, b_vocab = g["W_vocab"], g["b_vocab"]

    cap = np.asarray(captions)
    cap_in = cap[:, :T]
    cap_out = cap[:, 1:T + 1]
    x_emb = W_embed[cap_in]                      # [N, T, WD]
    xembT = np.ascontiguousarray(
        x_emb.transpose(2, 1, 0).reshape(WD, T * N)).astype(BF16)
    imgsT = np.ascontiguousarray(
        images.reshape(N, CIN, P).transpose(1, 0, 2).reshape(CIN, N * P)
    ).astype(BF16)
    mask = (cap_out != 0).astype(np.float32)     # [N, T]
    ident = np.eye(128, dtype=BF16)
    ones1 = np.ones((128, 1), dtype=BF16)
    onesrow = np.ones((1, 128), dtype=BF16)

    # attn projection fused with W_proj (B2 = imgs^T @ WPA + bb)
    WPA = (W_proj.astype(np.float32) @ Wattn.astype(np.float32))  # [CIN, 4H]
    bb = (b_proj.astype(np.float32) @ Wattn.astype(np.float32))   # [4H]

    # block-diag build constants
    rr = np.arange(128)
    nn_i = np.arange(128)
    maskt = (rr[None, :] // 16 == nn_i[:, None] % 8).astype(BF16)  # [n, r]
    kk = np.arange(16)
    sel = (nn_i[:, None] // 8 == kk[None, :]).astype(BF16)         # [n, k]
    jj = np.arange(32)
    mask32 = (jj[None, None, :] == 8 * (kk[None, :, None] % 4)
              + rr[:, None, None] // 16).astype(BF16)              # [r, k, j]
    mask32 = np.ascontiguousarray(mask32.reshape(128, 512))

    # label weight vectors, [H, N, T] -> per t: [hl, (n, kt)] (pre-halved)
    wy = W_vocab[:, cap_out] * 0.5               # [H, N, T]
    wy_t = wy.reshape(8, 128, N, T).transpose(3, 1, 2, 0)  # [T, hl, n, kt]

    in_maps = []
    for c in range(NCORES):
        hsl = slice(128 * c, 128 * (c + 1))
        idx = np.concatenate([g4 * H + 128 * c + np.arange(128)
                              for g4 in range(4)])
        vsl = slice(VS * c, VS * (c + 1))
        wyc = wy_t.copy()
        nm = np.zeros(N, dtype=wy_t.dtype)
        nm[16 * c:16 * (c + 1)] = 1
        wyc *= nm[None, None, :, None]
        in_maps.append({
            "imgsT": imgsT,
            "wprojS": np.ascontiguousarray(W_proj[:, hsl]).astype(BF16),
            "bprojR": np.ascontiguousarray(
                np.broadcast_to(b_proj[hsl], (N, HS))).astype(np.float32),
            "xembT": xembT,
            "WxS": np.ascontiguousarray(Wx[:, idx]).astype(BF16),
            "bSb": np.ascontiguousarray(
                np.broadcast_to(b[idx], (N, SC))).astype(np.float32),
            "WhS": np.ascontiguousarray(Wh[:, idx] * 0.5).astype(BF16),
            "wpaS": np.ascontiguousarray(WPA[:, idx]).astype(BF16),
            "bbrep": np.ascontiguousarray(
                np.broadcast_to(bb[idx], (N, SC))).astype(np.float32),
            "wvoc": np.ascontiguousarray(W_vocab[:, vsl] * 0.5).astype(BF16),
            "bvoc": np.ascontiguousarray(b_vocab[None, vsl]).astype(BF16),
            "wyT": np.ascontiguousarray(
                wyc.reshape(T, 128, H)).astype(BF16),
            "mask_f": mask,
            "ident": ident,
            "ones1": ones1,
            "onesrow": onesrow,
            "maskt": maskt,
            "sel": sel,
            "mask32": mask32,
        })
    host_by = float(np.sum(mask.astype(np.float64) *
                           np.asarray(b_vocab, np.float64)[cap_out]) / N)
    return in_maps, host_by


_CACHE = {}


def _get_built(T):
    if T not in _CACHE:
        _CACHE[T] = build(T)
    return _CACHE[T]


def run(inputs, T=30):
    nc = _get_built(T)
    in_maps, host_by = host_prep(inputs, T)
    res = run_bass_kernel_spmd(nc, in_maps, core_ids=list(range(NCORES)))
    dev_loss = float(res.results[0]["loss"][0, 0])
    return np.float32(dev_loss - host_by)


def kernel(**inputs) -> np.ndarray:
    return run(inputs, T=30)
